# revision 1
# baseline (speedup 1.0000x reference)
"""MoE (DeepSeek-style) routed+shared expert forward on 8 TRN2 NeuronCores.

Strategy (expert-parallel, host-side dispatch):
  - Host computes the gate (softmax + top-2) in float64 and gathers each
    expert's routed tokens (this is the "all-to-all by routing index" --
    with full inputs on the host, the host does the dispatch).
  - Core e processes expert e's routed tokens (padded to a uniform
    capacity C) through the SwiGLU FFN, plus a 1/8 slice of all tokens
    through the replicated shared-expert MLP.
  - All activations/weights are fed transposed (features on SBUF
    partitions, tokens on the free dim) so the w1/w3 -> swiglu -> w2
    chain needs no on-chip transposes.
  - Matmuls use float32r (full-rate fp32 on the PE array).
  - Host scatters expert outputs back by routing index, scales by the
    gate weights, and adds the shared-expert output.
"""

import sys

if "/opt/trn_rl_repo" not in sys.path:
    sys.path.insert(0, "/opt/trn_rl_repo")

import ml_dtypes
import numpy as np

import concourse.bass as bass
import concourse.tile as tile
from concourse import bacc, mybir
from concourse import bass_utils

B, S, DIM = 4, 2048, 1024
T = B * S
INTER = 1024
E = 8
TOPK = 2
ROUTE_SCALE = 1.0
SHARED_INTER = 2048
N_CORES = 8
TOKS_SHARED = T // N_CORES  # shared-expert tokens per core
BLK = 512

F32 = mybir.dt.float32
F32R = mybir.dt.float32r
BF16 = mybir.dt.bfloat16
SILU = mybir.ActivationFunctionType.Silu
IDENT = mybir.ActivationFunctionType.Identity

_program_cache = {}


def _blocks(total):
    """Split `total` columns into blocks of 512, keeping every block
    >= 256 (fp32r matmuls drop to 1/4 rate below 256): a short tail
    is merged with the previous 512 and split into two halves."""
    assert total >= 256
    sizes = []
    rem = total
    while rem > 0:
        if rem >= BLK + 256 or rem <= BLK:
            n = min(BLK, rem)
            if n < 256:  # tail < 256: merge with previous block
                n2 = sizes.pop() + n
                h = (n2 // 2) & ~1
                sizes.extend([h, n2 - h])
                rem = 0
                continue
            sizes.append(n)
            rem -= n
        else:  # 513..767 left: split into two even halves >= 256
            h = (rem // 2) & ~1
            sizes.extend([h, rem - h])
            rem = 0
    out, o = [], 0
    for n in sizes:
        out.append((o, n))
        o += n
    return out


def build_program(C):
    """Build the per-core SPMD Bass program for routed capacity C.

    Phase 1 (routed expert): fp32r matmuls, w1/w3/w2 resident in SBUF.
    Phase 2 (shared expert): bf16 matmuls, ws1/ws3/ws2 resident in SBUF,
    tokens processed in two halves of 512. Each weight/activation chunk
    is a separate tile so matmuls depend only on the chunk they read;
    DMA issue order matches PE consumption order.
    """
    nc = bacc.Bacc("TRN2", target_bir_lowering=False, debug=False,
                   num_devices=N_CORES)

    def din(name, shape, dt=F32):
        return nc.dram_tensor(name, shape, dt, kind="ExternalInput").ap()

    def dout(name, shape):
        return nc.dram_tensor(name, shape, F32, kind="ExternalOutput").ap()

    xe = din("xe", (DIM, C), F32R)            # routed tokens, transposed
    xs = din("xs", (DIM, TOKS_SHARED), BF16)  # shared-token slice, transposed
    w1t = din("w1t", (DIM, INTER), F32R)      # w1[e].T
    w3t = din("w3t", (DIM, INTER), F32R)
    w2t = din("w2t", (INTER, DIM), F32R)      # w2[e].T
    ws1t = din("ws1t", (DIM, SHARED_INTER), BF16)
    ws3t = din("ws3t", (DIM, SHARED_INTER), BF16)
    ws2t = din("ws2t", (SHARED_INTER, DIM), BF16)
    biases = din("biases", (128, 64))  # host-packed per-partition biases
    ye = dout("ye", (DIM, C))
    ys = dout("ys", (DIM, TOKS_SHARED))

    ND = DIM // 128           # 8 k-tiles over DIM
    NI = INTER // 128         # 8 tiles over INTER
    NS = SHARED_INTER // 128  # 16 tiles over SHARED_INTER

    xe_r = xe.rearrange("(dk p) c -> p dk c", p=128)
    xs_r = xs.rearrange("(dk p) c -> p dk c", p=128)
    w1_r = w1t.rearrange("(dk p) i -> p dk i", p=128)
    w3_r = w3t.rearrange("(dk p) i -> p dk i", p=128)
    w2_r = w2t.rearrange("(mi p) d -> p mi d", p=128)
    ws1_r = ws1t.rearrange("(dk p) i -> p dk i", p=128)
    ws3_r = ws3t.rearrange("(dk p) i -> p dk i", p=128)
    ws2_r = ws2t.rearrange("(mi p) d -> p mi d", p=128)
    ye_r = ye.rearrange("(md p) c -> p md c", p=128)
    ys_r = ys.rearrange("(md p) c -> p md c", p=128)

    with tile.TileContext(nc) as tc:
        with tc.tile_pool(name="bias", bufs=1) as bpool, \
             tc.tile_pool(name="tmp", bufs=2) as tpool, \
             tc.tile_pool(name="yout", bufs=2) as ypool, \
             tc.tile_pool(name="ps", bufs=2, space="PSUM") as pspool:
            ball = bpool.tile([128, 64], F32, tag="biases")
            nc.sync.dma_start(ball[:], biases[:])
            b1_sb = ball[:, 0:NI]
            b3_sb = ball[:, NI:2 * NI]
            b2_sb = ball[:, 2 * NI:2 * NI + ND]
            bs1_sb = ball[:, 24:24 + NS]
            bs3_sb = ball[:, 24 + NS:24 + 2 * NS]
            bs2_sb = ball[:, 24 + 2 * NS:24 + 2 * NS + ND]

            blocks = _blocks(C)
            nb = len(blocks)

            def load_xb(xpool, off, n):
                xb = []
                for dk in range(ND):
                    t = xpool.tile([128, n], F32R, tag=f"xb{dk}",
                                   name=f"xb{dk}", padded_shape=[128, BLK])
                    nc.sync.dma_start(t[:], xe_r[:, dk, off:off + n])
                    xb.append(t)
                return xb

            def mi_stage(w1_sb, w3_sb, xb, hb, n, width, nk, t1b, t3b, hoff=0):
                nm = width // 128
                for mi in range(nm):
                    ps1 = pspool.tile([128, n], F32, tag="ps1",
                                      padded_shape=[128, BLK])
                    ps3 = pspool.tile([128, n], F32, tag="ps3",
                                      padded_shape=[128, BLK])
                    for dk in range(nk):
                        nc.tensor.matmul(
                            ps1[:], w1_sb[dk][:, mi * 128:(mi + 1) * 128],
                            xb[dk][:, hoff:hoff + n],
                            start=(dk == 0), stop=(dk == nk - 1))
                    for dk in range(nk):
                        nc.tensor.matmul(
                            ps3[:], w3_sb[dk][:, mi * 128:(mi + 1) * 128],
                            xb[dk][:, hoff:hoff + n],
                            start=(dk == 0), stop=(dk == nk - 1))
                    hdt = hb[mi].dtype
                    tdt = BF16 if hdt == BF16 else F32
                    t1 = tpool.tile([128, n], tdt, tag=f"t1{tdt}",
                                    name="t1", padded_shape=[128, BLK])
                    nc.scalar.activation(t1[:], ps1[:], SILU,
                                         bias=t1b[:, mi:mi + 1])
                    t3 = tpool.tile([128, n], tdt, tag=f"t3{tdt}",
                                    name="t3", padded_shape=[128, BLK])
                    nc.scalar.activation(t3[:], ps3[:], IDENT,
                                         bias=t3b[:, mi:mi + 1])
                    nc.vector.tensor_mul(hb[mi][:], t1[:], t3[:])

            def md_stage(w2_sb, hb, out_r, off, n, nmi, b2b):
                for md in range(ND):
                    psy = pspool.tile([128, n], F32, tag="psy",
                                      padded_shape=[128, BLK])
                    for mi in range(nmi):
                        nc.tensor.matmul(
                            psy[:], w2_sb[mi][:, md * 128:(md + 1) * 128],
                            hb[mi][:],
                            start=(mi == 0), stop=(mi == nmi - 1))
                    yt = ypool.tile([128, n], F32, tag="yt",
                                    name="yt", padded_shape=[128, BLK])
                    nc.scalar.activation(yt[:], psy[:], IDENT,
                                         bias=b2b[:, md:md + 1])
                    nc.sync.dma_start(out_r[:, md, off:off + n], yt[:])

            # ---------- Phase 1: routed expert (fp32r, weights resident) ----
            xs_sb = []
            xsp = tc_xs_pool = None
            from contextlib import ExitStack
            es_xs = ExitStack()
            xsp = es_xs.enter_context(
                tc.tile_pool(name="xsp", bufs=1, side="right"))
            with tc.tile_pool(name="wexp", bufs=1) as wpool, \
                 tc.tile_pool(name="xbp", bufs=2) as xpool, \
                 tc.tile_pool(name="hbp", bufs=1) as hpool:
                xb0 = load_xb(xpool, 0, blocks[0][1])
                w1_sb, w3_sb, w2_sb = [], [], []
                for dk in range(ND):
                    t = wpool.tile([128, INTER], F32R, tag=f"w1_{dk}")
                    nc.sync.dma_start(t[:], w1_r[:, dk, :])
                    w1_sb.append(t)
                for dk in range(ND):
                    t = wpool.tile([128, INTER], F32R, tag=f"w3_{dk}")
                    nc.sync.dma_start(t[:], w3_r[:, dk, :])
                    w3_sb.append(t)
                for mi in range(NI):
                    t = wpool.tile([128, DIM], F32R, tag=f"w2_{mi}")
                    nc.sync.dma_start(t[:], w2_r[:, mi, :])
                    w2_sb.append(t)
                for dk in range(ND):
                    t = xsp.tile([128, TOKS_SHARED], BF16, tag=f"xs{dk}",
                                 name=f"xs{dk}")
                    nc.sync.dma_start(t[:], xs_r[:, dk, :])
                    xs_sb.append(t)

                for bi, (off, n) in enumerate(blocks):
                    xb = xb0 if bi == 0 else load_xb(xpool, off, n)
                    hb = [hpool.tile([128, n], F32R, tag=f"hb{mi}",
                                     name=f"hb{mi}", padded_shape=[128, BLK])
                          for mi in range(NI)]
                    mi_stage(w1_sb, w3_sb, xb, hb, n, INTER, ND, b1_sb, b3_sb)
                    md_stage(w2_sb, hb, ye_r, off, n, NI, b2_sb)

            # ---------- Phase 2: shared expert (bf16, weights resident) -----
            with tc.tile_pool(name="wsh", bufs=1) as wspool, \
                 tc.tile_pool(name="hsp", bufs=1) as hspool:
                ws1_sb, ws3_sb, ws2_sb = [], [], []
                for dk in range(ND):
                    t = wspool.tile([128, SHARED_INTER], BF16, tag=f"ws1_{dk}")
                    nc.sync.dma_start(t[:], ws1_r[:, dk, :])
                    ws1_sb.append(t)
                    t = wspool.tile([128, SHARED_INTER], BF16, tag=f"ws3_{dk}")
                    nc.sync.dma_start(t[:], ws3_r[:, dk, :])
                    ws3_sb.append(t)
                for mi in range(NS):
                    t = wspool.tile([128, DIM], BF16, tag=f"ws2_{mi}")
                    nc.sync.dma_start(t[:], ws2_r[:, mi, :])
                    ws2_sb.append(t)

                for (off, n) in _blocks(TOKS_SHARED):
                    hs = [hspool.tile([128, n], BF16, tag=f"hs{mi}",
                                      name=f"hs{mi}", padded_shape=[128, BLK])
                          for mi in range(NS)]
                    mi_stage(ws1_sb, ws3_sb, xs_sb, hs, n, SHARED_INTER, ND,
                             bs1_sb, bs3_sb, hoff=off)
                    md_stage(ws2_sb, hs, ys_r, off, n, NS, bs2_sb)
            es_xs.close()

    nc.compile()
    return nc


def _pack_biases(b1, b3, b2, bs1, bs3, bs2):
    """Pack all bias vectors into one [128, 64] per-partition table."""
    out = np.zeros((128, 64), np.float32)
    cols = [(b1, 0), (b3, 8), (b2, 16), (bs1, 24), (bs3, 40), (bs2, 56)]
    for vec, c0 in cols:
        k = len(vec) // 128
        out[:, c0:c0 + k] = vec.reshape(k, 128).T
    return out


def _gate_host(xt, gate_w, gate_b):
    """Softmax gate + top-2 routing, computed in float64 on the host."""
    logits = xt.astype(np.float64) @ gate_w.astype(np.float64).T \
        + gate_b.astype(np.float64)
    m = logits.max(axis=-1, keepdims=True)
    p = np.exp(logits - m)
    scores = p / p.sum(axis=-1, keepdims=True)
    order = np.argsort(-scores, axis=1, kind="stable")
    top_i = order[:, :TOPK]
    top_w = (np.take_along_axis(scores, top_i, axis=1)
             * ROUTE_SCALE).astype(np.float32)
    return top_i, top_w


def run(inputs, trace=False):
    x = np.ascontiguousarray(np.asarray(inputs["x"], dtype=np.float32))
    gate_w = np.asarray(inputs["gate_w"], dtype=np.float32)
    gate_b = np.asarray(inputs["gate_b"], dtype=np.float32)
    w1 = np.asarray(inputs["w1"], dtype=np.float32)
    b1 = np.asarray(inputs["b1"], dtype=np.float32)
    w3 = np.asarray(inputs["w3"], dtype=np.float32)
    b3 = np.asarray(inputs["b3"], dtype=np.float32)
    w2 = np.asarray(inputs["w2"], dtype=np.float32)
    b2 = np.asarray(inputs["b2"], dtype=np.float32)
    ws1 = np.asarray(inputs["ws1"], dtype=np.float32)
    bs1 = np.asarray(inputs["bs1"], dtype=np.float32)
    ws3 = np.asarray(inputs["ws3"], dtype=np.float32)
    bs3 = np.asarray(inputs["bs3"], dtype=np.float32)
    ws2 = np.asarray(inputs["ws2"], dtype=np.float32)
    bs2 = np.asarray(inputs["bs2"], dtype=np.float32)

    xt = x.reshape(T, DIM)
    top_i, top_w = _gate_host(xt, gate_w, gate_b)

    # Dispatch: token lists + gate weights per expert.
    idx, wgt = [], []
    for e in range(E):
        toks = np.nonzero((top_i == e).any(axis=1))[0]
        idx.append(toks)
        slot = (top_i[toks] == e)            # [n_e, TOPK], exactly one True/row
        wgt.append(top_w[toks][slot])

    cmax = max(len(i) for i in idx)
    C = max(256, cmax + (cmax & 1))   # fp32r matmul needs an even free dim

    bf16 = ml_dtypes.bfloat16
    ws1t = np.ascontiguousarray(ws1.T).astype(bf16)
    ws3t = np.ascontiguousarray(ws3.T).astype(bf16)
    ws2t = np.ascontiguousarray(ws2.T).astype(bf16)
    xt_bf = xt.astype(bf16)

    in_maps = []
    for e in range(E):
        xe = np.zeros((DIM, C), np.float32)
        xe[:, :len(idx[e])] = xt[idx[e]].T
        sl = slice(TOKS_SHARED * e, TOKS_SHARED * (e + 1))
        in_maps.append({
            "xe": xe,
            "xs": np.ascontiguousarray(xt_bf[sl].T),
            "w1t": np.ascontiguousarray(w1[e].T),
            "w3t": np.ascontiguousarray(w3[e].T),
            "w2t": np.ascontiguousarray(w2[e].T),
            "ws1t": ws1t, "ws3t": ws3t, "ws2t": ws2t,
            "biases": _pack_biases(b1[e], b3[e], b2[e], bs1, bs3, bs2),
        })

    if C not in _program_cache:
        _program_cache[C] = build_program(C)
    nc = _program_cache[C]

    res = bass_utils.run_bass_kernel_spmd(
        nc, in_maps, core_ids=list(range(N_CORES)), trace=trace)

    y = np.empty((T, DIM), np.float32)
    for e in range(E):
        sl = slice(TOKS_SHARED * e, TOKS_SHARED * (e + 1))
        y[sl] = res.results[e]["ys"].T
    for e in range(E):
        ye = res.results[e]["ye"]
        y[idx[e]] += ye[:, :len(idx[e])].T * wgt[e][:, None]
    return y.reshape(B, S, DIM), res


def kernel(**inputs) -> np.ndarray:
    out, _ = run(inputs, trace=False)
    return out



# revision 2
# speedup vs baseline: 1.3305x; 1.3305x over previous
"""MoE (DeepSeek-style) routed+shared expert forward on 8 TRN2 NeuronCores.

Strategy (expert-parallel, host-side dispatch):
  - Host computes the gate (softmax + top-2) in float64 and gathers each
    expert's routed tokens; core e processes expert e's tokens (padded to
    capacity C) plus a 1/8 slice of all tokens through the replicated
    shared-expert MLP.
  - Routed expert runs in fp8(e4m3) with DoubleRow matmuls (2x PE rate).
    Host quantizes x and the expert weights with per-tensor scales; the
    dequant scales fold into the activation/vector ops. Error budget: the
    routed path carries only ~23% of the output norm (gate weights are
    softmax scores ~0.2), so fp8's ~6% relative error lands at ~1.5e-2
    overall, within the 2e-2 tolerance.
  - Shared expert (97% of the output norm) stays bf16.
  - All weights are SBUF-resident from the start (fp8 routed weights are
    tiny); DMA issue order matches PE consumption order so the PE starts
    ~7us in and never waits on the phase transition.
  - A short burst of warmup matmuls on scratch SBUF runs during the input
    DMA so the PE's DVFS clock is ramped when real work arrives.
"""

import sys

if "/opt/trn_rl_repo" not in sys.path:
    sys.path.insert(0, "/opt/trn_rl_repo")

import ml_dtypes
import numpy as np

import concourse.bass as bass
import concourse.tile as tile
from concourse import bacc, mybir
from concourse import bass_utils
from concourse.alu_op_type import AluOpType

B, S, DIM = 4, 2048, 1024
T = B * S
INTER = 1024
E = 8
TOPK = 2
ROUTE_SCALE = 1.0
SHARED_INTER = 2048
N_CORES = 8
TS = T // N_CORES  # shared-expert tokens per core
BLK = 512
N_WARM = 24

F32 = mybir.dt.float32
BF16 = mybir.dt.bfloat16
FP8 = mybir.dt.float8e4
SILU = mybir.ActivationFunctionType.Silu
IDENT = mybir.ActivationFunctionType.Identity
DR = mybir.MatmulPerfMode.DoubleRow
MUL = AluOpType.mult
ADD = AluOpType.add

E4NP = ml_dtypes.float8_e4m3fn
BFNP = ml_dtypes.bfloat16

ND = DIM // 128           # 8 k-tiles over DIM
NP = ND // 2              # 4 DoubleRow k-pair tiles over DIM
NI = INTER // 128         # 8 tiles over INTER
NS = SHARED_INTER // 128  # 16 tiles over SHARED_INTER

_program_cache = {}


def _blocks(total):
    full = total // BLK
    out = [(i * BLK, BLK) for i in range(full)]
    if total - full * BLK:
        out.append((full * BLK, total - full * BLK))
    return out


def build_program(C):
    nc = bacc.Bacc("TRN2", target_bir_lowering=False, debug=False,
                   num_devices=N_CORES)

    def din(name, shape, dt):
        return nc.dram_tensor(name, shape, dt, kind="ExternalInput").ap()

    def dout(name, shape, dt):
        return nc.dram_tensor(name, shape, dt, kind="ExternalOutput").ap()

    xe8 = din("xe8", (NP, 128, 2, C), FP8)       # routed tokens, fp8 pairs
    w18 = din("w18", (NP, 128, 2, INTER), FP8)   # w1[e].T in DR pair layout
    w38 = din("w38", (NP, 128, 2, INTER), FP8)
    w28 = din("w28", (NP, 128, 2, DIM), FP8)     # w2[e].T in DR pair layout
    xs = din("xs", (ND, 128, TS), BF16)          # shared-token slice
    ws1 = din("ws1", (ND, 128, SHARED_INTER), BF16)
    ws3 = din("ws3", (ND, 128, SHARED_INTER), BF16)
    ws2 = din("ws2", (NS, 128, DIM), BF16)
    scb = din("scb", (128, 68), F32)  # packed biases + dequant scales
    ye = dout("ye", (ND, 128, C), BF16)
    ys = dout("ys", (ND, 128, TS), F32)

    rblocks = _blocks(C)
    sblocks = _blocks(TS)

    with tile.TileContext(nc) as tc:
        with tc.tile_pool(name="const", bufs=1) as cpool, \
             tc.tile_pool(name="tmp", bufs=2) as tpool, \
             tc.tile_pool(name="hr", bufs=2) as hrpool, \
             tc.tile_pool(name="hsh", bufs=2) as hspool, \
             tc.tile_pool(name="yout", bufs=2) as ypool, \
             tc.tile_pool(name="ps", bufs=2, space="PSUM") as pspool, \
             tc.tile_pool(name="wps", bufs=1, space="PSUM") as wpspool:

            # ---- PE warmup: ramp the DVFS clock while input DMA runs ----
            wsc = cpool.tile([128, 256], BF16, tag="warm")
            nc.vector.memset(wsc[:], 0.25)
            wps = wpspool.tile([128, 256], F32, tag="wps")
            for _ in range(N_WARM):
                nc.tensor.matmul(wps[:], wsc[:, 0:128], wsc[:],
                                 start=True, stop=True)

            # ---- input DMAs, issued in PE consumption order ----
            ball = cpool.tile([128, 68], F32, tag="scb")
            nc.sync.dma_start(ball[:], scb[:])
            b1c = lambda mi: ball[:, mi:mi + 1]
            b3c = lambda mi: ball[:, 8 + mi:9 + mi]
            b2c = lambda md: ball[:, 16 + md:17 + md]
            bs1c = lambda mi: ball[:, 24 + mi:25 + mi]
            bs3c = lambda mi: ball[:, 40 + mi:41 + mi]
            bs2c = lambda md: ball[:, 56 + md:57 + md]
            sc1 = ball[:, 64:65]
            sc3 = ball[:, 65:66]
            scy = ball[:, 66:67]

            w1_t, w3_t, w2_t, xe_t = [], [], [], []
            for j in range(NP):
                t = cpool.tile([128, 2, INTER], FP8, tag=f"w1_{j}")
                nc.sync.dma_start(t[:], w18[j])
                w1_t.append(t)
            for j in range(NP):
                t = cpool.tile([128, 2, C], FP8, tag=f"xe_{j}", name=f"xe{j}")
                xe_t.append(t)
            for (off, n) in rblocks:  # chunked so block 0 can start early
                for j in range(NP):
                    nc.sync.dma_start(xe_t[j][:, :, off:off + n],
                                      xe8[j][:, :, off:off + n])
            for j in range(NP):
                t = cpool.tile([128, 2, INTER], FP8, tag=f"w3_{j}")
                nc.sync.dma_start(t[:], w38[j])
                w3_t.append(t)
            for j in range(NP):
                t = cpool.tile([128, 2, DIM], FP8, tag=f"w2_{j}")
                nc.sync.dma_start(t[:], w28[j])
                w2_t.append(t)
            xs_t = []
            for dk in range(ND):
                t = cpool.tile([128, TS], BF16, tag=f"xs_{dk}", name=f"xs{dk}")
                nc.sync.dma_start(t[:], xs[dk])
                xs_t.append(t)
            ws1_t, ws3_t, ws2_t = [], [], []
            for dk in range(ND):
                t = cpool.tile([128, SHARED_INTER], BF16, tag=f"ws1_{dk}")
                nc.sync.dma_start(t[:], ws1[dk])
                ws1_t.append(t)
            for dk in range(ND):
                t = cpool.tile([128, SHARED_INTER], BF16, tag=f"ws3_{dk}")
                nc.sync.dma_start(t[:], ws3[dk])
                ws3_t.append(t)
            for mi in range(NS):
                t = cpool.tile([128, DIM], BF16, tag=f"ws2_{mi}")
                nc.sync.dma_start(t[:], ws2[mi])
                ws2_t.append(t)

            # ---- Phase 1: routed expert, fp8 DoubleRow ----
            for (off, n) in rblocks:
                h8 = [hrpool.tile([128, 2, n], FP8, tag=f"h8_{j}",
                                  name=f"h8{j}", padded_shape=[128, 2, BLK])
                      for j in range(NP)]
                for j in range(NP):
                    for i in range(2):
                        mi = 2 * j + i
                        ps1 = pspool.tile([128, n], F32, tag="ps1",
                                          padded_shape=[128, BLK])
                        for jj in range(NP):
                            nc.tensor.matmul(
                                ps1[:], w1_t[jj][:, :, mi * 128:(mi + 1) * 128],
                                xe_t[jj][:, :, off:off + n],
                                start=(jj == 0), stop=(jj == NP - 1),
                                perf_mode=DR)
                        ps3 = pspool.tile([128, n], F32, tag="ps3",
                                          padded_shape=[128, BLK])
                        for jj in range(NP):
                            nc.tensor.matmul(
                                ps3[:], w3_t[jj][:, :, mi * 128:(mi + 1) * 128],
                                xe_t[jj][:, :, off:off + n],
                                start=(jj == 0), stop=(jj == NP - 1),
                                perf_mode=DR)
                        t1 = tpool.tile([128, n], BF16, tag="t1", name="t1",
                                        padded_shape=[128, BLK])
                        nc.scalar.activation(t1[:], ps1[:], SILU,
                                             bias=b1c(mi), scale=sc1)
                        t3 = tpool.tile([128, n], BF16, tag="t3", name="t3",
                                        padded_shape=[128, BLK])
                        nc.vector.tensor_scalar(t3[:], ps3[:], sc3, b3c(mi),
                                                MUL, ADD)
                        nc.vector.tensor_mul(h8[j][:, i, :], t1[:], t3[:])
                for md in range(ND):
                    psy = pspool.tile([128, n], F32, tag="psy",
                                      padded_shape=[128, BLK])
                    for j in range(NP):
                        nc.tensor.matmul(
                            psy[:], w2_t[j][:, :, md * 128:(md + 1) * 128],
                            h8[j][:, :, :],
                            start=(j == 0), stop=(j == NP - 1),
                            perf_mode=DR)
                    yt = ypool.tile([128, n], BF16, tag="yt", name="yt",
                                    padded_shape=[128, BLK])
                    nc.scalar.activation(yt[:], psy[:], IDENT,
                                         bias=b2c(md), scale=scy)
                    nc.sync.dma_start(ye[md][:, off:off + n], yt[:])

            # ---- Phase 2: shared expert, bf16 ----
            for (off, n) in sblocks:
                hs = [hspool.tile([128, n], BF16, tag=f"hs_{mi}",
                                  name=f"hs{mi}", padded_shape=[128, BLK])
                      for mi in range(NS)]
                for mi in range(NS):
                    ps1 = pspool.tile([128, n], F32, tag="ps1",
                                      padded_shape=[128, BLK])
                    for dk in range(ND):
                        nc.tensor.matmul(
                            ps1[:], ws1_t[dk][:, mi * 128:(mi + 1) * 128],
                            xs_t[dk][:, off:off + n],
                            start=(dk == 0), stop=(dk == ND - 1))
                    ps3 = pspool.tile([128, n], F32, tag="ps3",
                                      padded_shape=[128, BLK])
                    for dk in range(ND):
                        nc.tensor.matmul(
                            ps3[:], ws3_t[dk][:, mi * 128:(mi + 1) * 128],
                            xs_t[dk][:, off:off + n],
                            start=(dk == 0), stop=(dk == ND - 1))
                    t1 = tpool.tile([128, n], BF16, tag="t1", name="t1",
                                    padded_shape=[128, BLK])
                    nc.scalar.activation(t1[:], ps1[:], SILU, bias=bs1c(mi))
                    t3 = tpool.tile([128, n], BF16, tag="t3", name="t3",
                                    padded_shape=[128, BLK])
                    nc.vector.tensor_scalar(t3[:], ps3[:], 1.0, bs3c(mi),
                                            MUL, ADD)
                    nc.vector.tensor_mul(hs[mi][:], t1[:], t3[:])
                for md in range(ND):
                    psy = pspool.tile([128, n], F32, tag="psy",
                                      padded_shape=[128, BLK])
                    for mi in range(NS):
                        nc.tensor.matmul(
                            psy[:], ws2_t[mi][:, md * 128:(md + 1) * 128],
                            hs[mi][:],
                            start=(mi == 0), stop=(mi == NS - 1))
                    yts = ypool.tile([128, n], F32, tag="yts", name="yts",
                                     padded_shape=[128, BLK])
                    nc.scalar.activation(yts[:], psy[:], IDENT, bias=bs2c(md))
                    nc.sync.dma_start(ys[md][:, off:off + n], yts[:])

    nc.compile()
    return nc


def _q8(a):
    return np.clip(a, -448.0, 448.0).astype(E4NP)


def _pack_w(w, scale):
    """[out, K] weight -> [K/256, 128, 2, out] fp8 DoubleRow pair layout."""
    K = w.shape[1]
    A = (w.T * scale).reshape(K // 256, 2, 128, w.shape[0])
    return _q8(np.ascontiguousarray(A.transpose(0, 2, 1, 3)))


def _pack_x(xg, scale, C):
    """[n, DIM] tokens -> [DIM/256, 128, 2, C] fp8 DoubleRow pair layout."""
    A = np.zeros((DIM, C), np.float32)
    A[:, :xg.shape[0]] = (xg * scale).T
    A = A.reshape(NP, 2, 128, C)
    return _q8(np.ascontiguousarray(A.transpose(0, 2, 1, 3)))


def _gate_host(xt, gate_w, gate_b):
    logits = xt.astype(np.float64) @ gate_w.astype(np.float64).T \
        + gate_b.astype(np.float64)
    m = logits.max(axis=-1, keepdims=True)
    p = np.exp(logits - m)
    scores = p / p.sum(axis=-1, keepdims=True)
    order = np.argsort(-scores, axis=1, kind="stable")
    top_i = order[:, :TOPK]
    top_w = (np.take_along_axis(scores, top_i, axis=1)
             * ROUTE_SCALE).astype(np.float32)
    return top_i, top_w


def run(inputs, trace=False):
    x = np.ascontiguousarray(np.asarray(inputs["x"], dtype=np.float32))
    gate_w = np.asarray(inputs["gate_w"], dtype=np.float32)
    gate_b = np.asarray(inputs["gate_b"], dtype=np.float32)
    w1 = np.asarray(inputs["w1"], dtype=np.float32)
    b1 = np.asarray(inputs["b1"], dtype=np.float32)
    w3 = np.asarray(inputs["w3"], dtype=np.float32)
    b3 = np.asarray(inputs["b3"], dtype=np.float32)
    w2 = np.asarray(inputs["w2"], dtype=np.float32)
    b2 = np.asarray(inputs["b2"], dtype=np.float32)
    ws1 = np.asarray(inputs["ws1"], dtype=np.float32)
    bs1 = np.asarray(inputs["bs1"], dtype=np.float32)
    ws3 = np.asarray(inputs["ws3"], dtype=np.float32)
    bs3 = np.asarray(inputs["bs3"], dtype=np.float32)
    ws2 = np.asarray(inputs["ws2"], dtype=np.float32)
    bs2 = np.asarray(inputs["bs2"], dtype=np.float32)

    xt = x.reshape(T, DIM)
    top_i, top_w = _gate_host(xt, gate_w, gate_b)

    idx, wgt = [], []
    for e in range(E):
        toks = np.nonzero((top_i == e).any(axis=1))[0]
        idx.append(toks)
        slot = (top_i[toks] == e)
        wgt.append(top_w[toks][slot])

    cmax = max(len(i) for i in idx)
    C = max(256, -(-cmax // 32) * 32)

    # fp8 scales: per-tensor for x, per-expert per-tensor for weights; the
    # h scale comes from a 32-token fp32 sample of the true h distribution.
    sx = 16.0 / max(xt.std(), 1e-30)
    xprobe = xt[:32]
    s1 = np.empty(E, np.float64); s3 = np.empty(E, np.float64)
    s2 = np.empty(E, np.float64); sh = np.empty(E, np.float64)
    for e in range(E):
        s1[e] = 16.0 / max(w1[e].std(), 1e-30)
        s3[e] = 16.0 / max(w3[e].std(), 1e-30)
        s2[e] = 16.0 / max(w2[e].std(), 1e-30)
        a = xprobe @ w1[e].T + b1[e]
        bb = xprobe @ w3[e].T + b3[e]
        h = a / (1.0 + np.exp(-a)) * bb
        sh[e] = 8.0 / max(h.std(), 1e-30)

    ws1p = np.ascontiguousarray(
        ws1.T.reshape(ND, 128, SHARED_INTER)).astype(BFNP)
    ws3p = np.ascontiguousarray(
        ws3.T.reshape(ND, 128, SHARED_INTER)).astype(BFNP)
    ws2p = np.ascontiguousarray(ws2.T.reshape(NS, 128, DIM)).astype(BFNP)

    in_maps = []
    for e in range(E):
        scbuf = np.zeros((128, 68), np.float32)
        scbuf[:, 0:8] = b1[e].reshape(8, 128).T
        scbuf[:, 8:16] = (b3[e] * sh[e]).reshape(8, 128).T
        scbuf[:, 16:24] = b2[e].reshape(8, 128).T
        scbuf[:, 24:40] = bs1.reshape(16, 128).T
        scbuf[:, 40:56] = bs3.reshape(16, 128).T
        scbuf[:, 56:64] = bs2.reshape(8, 128).T
        scbuf[:, 64] = 1.0 / (sx * s1[e])
        scbuf[:, 65] = sh[e] / (sx * s3[e])
        scbuf[:, 66] = 1.0 / (sh[e] * s2[e])
        sl = slice(TS * e, TS * (e + 1))
        in_maps.append({
            "xe8": _pack_x(xt[idx[e]], sx, C),
            "w18": _pack_w(w1[e], s1[e]),
            "w38": _pack_w(w3[e], s3[e]),
            "w28": _pack_w(w2[e], s2[e]),
            "xs": np.ascontiguousarray(
                xt[sl].T.reshape(ND, 128, TS)).astype(BFNP),
            "ws1": ws1p, "ws3": ws3p, "ws2": ws2p,
            "scb": scbuf,
        })

    if C not in _program_cache:
        _program_cache[C] = build_program(C)
    nc = _program_cache[C]

    res = bass_utils.run_bass_kernel_spmd(
        nc, in_maps, core_ids=list(range(N_CORES)), trace=trace)

    y = np.empty((T, DIM), np.float32)
    for e in range(E):
        sl = slice(TS * e, TS * (e + 1))
        y[sl] = res.results[e]["ys"].reshape(DIM, TS).T
    for e in range(E):
        yee = res.results[e]["ye"].reshape(DIM, C).astype(np.float32)
        y[idx[e]] += yee[:, :len(idx[e])].T * wgt[e][:, None]
    return y.reshape(B, S, DIM), res


def kernel(**inputs) -> np.ndarray:
    out, _ = run(inputs, trace=False)
    return out


# revision 6
# speedup vs baseline: 1.4687x; 1.1039x over previous
"""MoE (DeepSeek-style) routed+shared expert forward on 8 TRN2 NeuronCores.

Strategy (expert-parallel, host-side dispatch):
  - Host computes the gate (softmax + top-2) in float64 and gathers each
    expert's routed tokens; core e processes expert e's tokens (padded to
    capacity C) plus a 1/8 slice of all tokens through the replicated
    shared-expert MLP.
  - Routed expert runs in fp8(e4m3) with DoubleRow matmuls (2x PE rate).
    Host quantizes x and the expert weights with per-tensor scales; the
    dequant scales fold into the activation/vector ops. Error budget: the
    routed path carries only ~23% of the output norm (gate weights are
    softmax scores ~0.2), so fp8's ~6% relative error lands at ~1.5e-2
    overall, within the 2e-2 tolerance.
  - Shared expert (97% of the output norm) stays bf16.
  - All weights are SBUF-resident from the start (fp8 routed weights are
    tiny); DMA issue order matches PE consumption order so the PE starts
    ~7us in and never waits on the phase transition.
  - A short burst of warmup matmuls on scratch SBUF runs during the input
    DMA so the PE's DVFS clock is ramped when real work arrives.
"""

import sys

if "/opt/trn_rl_repo" not in sys.path:
    sys.path.insert(0, "/opt/trn_rl_repo")

import ml_dtypes
import numpy as np

import concourse.bass as bass
import concourse.tile as tile
from concourse import bacc, mybir
from concourse import bass_utils
from concourse.alu_op_type import AluOpType

B, S, DIM = 4, 2048, 1024
T = B * S
INTER = 1024
E = 8
TOPK = 2
ROUTE_SCALE = 1.0
SHARED_INTER = 2048
N_CORES = 8
TS = T // N_CORES  # shared-expert tokens per core
BLK = 512
N_WARM = 24

F32 = mybir.dt.float32
BF16 = mybir.dt.bfloat16
FP8 = mybir.dt.float8e4
SILU = mybir.ActivationFunctionType.Silu
IDENT = mybir.ActivationFunctionType.Identity
DR = mybir.MatmulPerfMode.DoubleRow
MUL = AluOpType.mult
ADD = AluOpType.add

E4NP = ml_dtypes.float8_e4m3fn
BFNP = ml_dtypes.bfloat16

ND = DIM // 128           # 8 k-tiles over DIM
NP = ND // 2              # 4 DoubleRow k-pair tiles over DIM
NI = INTER // 128         # 8 tiles over INTER
NS = SHARED_INTER // 128  # 16 tiles over SHARED_INTER

_program_cache = {}


def _blocks(total):
    """Split into <=512-wide even blocks of near-equal size (all >=256 so
    per-instruction LDWEIGHTS overhead stays hidden)."""
    nb = -(-total // BLK)
    b = -(-total // (nb * 32)) * 32
    sizes = [b] * (nb - 1) + [total - b * (nb - 1)]
    assert all(256 <= s <= BLK and s % 2 == 0 for s in sizes), sizes
    out, o = [], 0
    for s in sizes:
        out.append((o, s))
        o += s
    return out


def build_program(C):
    nc = bacc.Bacc("TRN2", target_bir_lowering=False, debug=False,
                   num_devices=N_CORES)

    def din(name, shape, dt):
        return nc.dram_tensor(name, shape, dt, kind="ExternalInput").ap()

    def dout(name, shape, dt):
        return nc.dram_tensor(name, shape, dt, kind="ExternalOutput").ap()

    xe8 = din("xe8", (NP, 128, 2, C), FP8)       # routed tokens, fp8 pairs
    w18 = din("w18", (NP, 128, 2, INTER), FP8)   # w1[e].T in DR pair layout
    w38 = din("w38", (NP, 128, 2, INTER), FP8)
    w28 = din("w28", (NP, 128, 2, DIM), FP8)     # w2[e].T in DR pair layout
    xs = din("xs", (ND, 128, TS), BF16)          # shared-token slice
    ws1 = din("ws1", (ND, 128, SHARED_INTER), BF16)
    ws3 = din("ws3", (ND, 128, SHARED_INTER), BF16)
    ws2 = din("ws2", (NS, 128, DIM), BF16)
    scb = din("scb", (128, 68), F32)  # packed biases + dequant scales
    ye = dout("ye", (ND, 128, C), BF16)
    ys = dout("ys", (ND, 128, TS), F32)

    rblocks = _blocks(C)
    sblocks = _blocks(TS)

    with tile.TileContext(nc) as tc:
        with tc.tile_pool(name="const", bufs=1) as cpool, \
             tc.tile_pool(name="tmp", bufs=2) as tpool, \
             tc.tile_pool(name="hr", bufs=2) as hrpool, \
             tc.tile_pool(name="hsh", bufs=2) as hspool, \
             tc.tile_pool(name="yout", bufs=4) as ypool, \
             tc.tile_pool(name="ysout", bufs=2) as yspool, \
             tc.tile_pool(name="ps", bufs=2, space="PSUM") as pspool, \
             tc.tile_pool(name="wps", bufs=1, space="PSUM") as wpspool:

            # ---- PE warmup: ramp the DVFS clock while input DMA runs ----
            wsc = cpool.tile([128, 256], BF16, tag="warm")
            nc.vector.memset(wsc[:], 0.25)
            wps = wpspool.tile([128, 256], F32, tag="wps")
            for _ in range(N_WARM):
                nc.tensor.matmul(wps[:], wsc[:, 0:128], wsc[:],
                                 start=True, stop=True)

            # ---- input DMAs, issued in PE consumption order ----
            ball = cpool.tile([128, 68], F32, tag="scb")
            nc.sync.dma_start(ball[:], scb[:])
            b1c = lambda mi: ball[:, mi:mi + 1]
            b3c = lambda mi: ball[:, 8 + mi:9 + mi]
            b2c = lambda md: ball[:, 16 + md:17 + md]
            bs1c = lambda mi: ball[:, 24 + mi:25 + mi]
            bs3c = lambda mi: ball[:, 40 + mi:41 + mi]
            bs2c = lambda md: ball[:, 56 + md:57 + md]
            sc1 = ball[:, 64:65]
            sc3 = ball[:, 65:66]
            scy = ball[:, 66:67]

            w1_t, w3_t, w2_t, xe_t = [], [], [], []
            for j in range(NP):
                t = cpool.tile([128, 2, INTER], FP8, tag=f"w1_{j}")
                nc.sync.dma_start(t[:], w18[j])
                w1_t.append(t)
            for j in range(NP):
                t = cpool.tile([128, 2, C], FP8, tag=f"xe_{j}", name=f"xe{j}")
                xe_t.append(t)
            off0, n0 = rblocks[0]
            for j in range(NP):  # block 0's tokens first so the PE can start
                nc.sync.dma_start(xe_t[j][:, :, off0:off0 + n0],
                                  xe8[j][:, :, off0:off0 + n0])
            for j in range(NP):
                t = cpool.tile([128, 2, INTER], FP8, tag=f"w3_{j}")
                nc.sync.dma_start(t[:], w38[j])
                w3_t.append(t)
            for j in range(NP):
                t = cpool.tile([128, 2, DIM], FP8, tag=f"w2_{j}")
                nc.sync.dma_start(t[:], w28[j])
                w2_t.append(t)
            for (off, n) in rblocks[1:]:
                for j in range(NP):
                    nc.sync.dma_start(xe_t[j][:, :, off:off + n],
                                      xe8[j][:, :, off:off + n])
            xs_t = []
            for dk in range(ND):
                t = cpool.tile([128, TS], BF16, tag=f"xs_{dk}", name=f"xs{dk}")
                nc.sync.dma_start(t[:], xs[dk])
                xs_t.append(t)
            ws1_t, ws3_t, ws2_t = [], [], []
            for dk in range(ND):
                ws1_t.append(cpool.tile([128, SHARED_INTER], BF16,
                                        tag=f"ws1_{dk}", name=f"ws1{dk}"))
                ws3_t.append(cpool.tile([128, SHARED_INTER], BF16,
                                        tag=f"ws3_{dk}", name=f"ws3{dk}"))
            for mi in range(NS):
                ws2_t.append(cpool.tile([128, DIM], BF16, tag=f"ws2_{mi}",
                                        name=f"ws2{mi}"))
            # shared-weight DMAs interleaved with the routed block loop so
            # routed output DMAs are never queued behind a 12MB preload
            ws_dmas = ([(ws1_t[dk], ws1[dk]) for dk in range(ND)]
                       + [(ws3_t[dk], ws3[dk]) for dk in range(ND)]
                       + [(ws2_t[mi], ws2[mi]) for mi in range(NS)])
            nws = -(-len(ws_dmas) // len(rblocks))

            # ---- Phase 1: routed expert, fp8 DoubleRow ----
            for bi, (off, n) in enumerate(rblocks):
                for dst, src in ws_dmas[bi * nws:(bi + 1) * nws]:
                    nc.sync.dma_start(dst[:], src)
                h8 = [hrpool.tile([128, 2, n], FP8, tag=f"h8_{j}",
                                  name=f"h8{j}", padded_shape=[128, 2, BLK])
                      for j in range(NP)]
                for j in range(NP):
                    for i in range(2):
                        mi = 2 * j + i
                        ps1 = pspool.tile([128, n], F32, tag="ps1",
                                          padded_shape=[128, BLK])
                        for jj in range(NP):
                            nc.tensor.matmul(
                                ps1[:], w1_t[jj][:, :, mi * 128:(mi + 1) * 128],
                                xe_t[jj][:, :, off:off + n],
                                start=(jj == 0), stop=(jj == NP - 1),
                                perf_mode=DR)
                        ps3 = pspool.tile([128, n], F32, tag="ps3",
                                          padded_shape=[128, BLK])
                        for jj in range(NP):
                            nc.tensor.matmul(
                                ps3[:], w3_t[jj][:, :, mi * 128:(mi + 1) * 128],
                                xe_t[jj][:, :, off:off + n],
                                start=(jj == 0), stop=(jj == NP - 1),
                                perf_mode=DR)
                        t1 = tpool.tile([128, n], BF16, tag="t1", name="t1",
                                        padded_shape=[128, BLK])
                        nc.scalar.activation(t1[:], ps1[:], SILU,
                                             bias=b1c(mi), scale=sc1)
                        t3 = tpool.tile([128, n], BF16, tag="t3", name="t3",
                                        padded_shape=[128, BLK])
                        nc.vector.tensor_scalar(t3[:], ps3[:], sc3, b3c(mi),
                                                MUL, ADD)
                        nc.vector.tensor_mul(h8[j][:, i, :], t1[:], t3[:])
                for md in range(ND):
                    psy = pspool.tile([128, n], F32, tag="psy",
                                      padded_shape=[128, BLK])
                    for j in range(NP):
                        nc.tensor.matmul(
                            psy[:], w2_t[j][:, :, md * 128:(md + 1) * 128],
                            h8[j][:, :, :],
                            start=(j == 0), stop=(j == NP - 1),
                            perf_mode=DR)
                    yt = ypool.tile([128, n], BF16, tag="yt", name="yt",
                                    padded_shape=[128, BLK])
                    nc.scalar.activation(yt[:], psy[:], IDENT,
                                         bias=b2c(md), scale=scy)
                    nc.sync.dma_start(ye[md][:, off:off + n], yt[:])

            # ---- Phase 2: shared expert, bf16 ----
            for (off, n) in sblocks:
                hs = [hspool.tile([128, n], BF16, tag=f"hs_{mi}",
                                  name=f"hs{mi}", padded_shape=[128, BLK])
                      for mi in range(NS)]
                for mi in range(NS):
                    ps1 = pspool.tile([128, n], F32, tag="ps1",
                                      padded_shape=[128, BLK])
                    for dk in range(ND):
                        nc.tensor.matmul(
                            ps1[:], ws1_t[dk][:, mi * 128:(mi + 1) * 128],
                            xs_t[dk][:, off:off + n],
                            start=(dk == 0), stop=(dk == ND - 1))
                    ps3 = pspool.tile([128, n], F32, tag="ps3",
                                      padded_shape=[128, BLK])
                    for dk in range(ND):
                        nc.tensor.matmul(
                            ps3[:], ws3_t[dk][:, mi * 128:(mi + 1) * 128],
                            xs_t[dk][:, off:off + n],
                            start=(dk == 0), stop=(dk == ND - 1))
                    t1 = tpool.tile([128, n], BF16, tag="t1", name="t1",
                                    padded_shape=[128, BLK])
                    nc.scalar.activation(t1[:], ps1[:], SILU, bias=bs1c(mi))
                    t3 = tpool.tile([128, n], BF16, tag="t3", name="t3",
                                    padded_shape=[128, BLK])
                    nc.vector.tensor_scalar(t3[:], ps3[:], 1.0, bs3c(mi),
                                            MUL, ADD)
                    nc.vector.tensor_mul(hs[mi][:], t1[:], t3[:])
                for md in range(ND):
                    psy = pspool.tile([128, n], F32, tag="psy",
                                      padded_shape=[128, BLK])
                    for mi in range(NS):
                        nc.tensor.matmul(
                            psy[:], ws2_t[mi][:, md * 128:(md + 1) * 128],
                            hs[mi][:],
                            start=(mi == 0), stop=(mi == NS - 1))
                    yts = yspool.tile([128, n], F32, tag="yts", name="yts",
                                      padded_shape=[128, BLK])
                    nc.scalar.activation(yts[:], psy[:], IDENT, bias=bs2c(md))
                    nc.sync.dma_start(ys[md][:, off:off + n], yts[:])

    nc.compile()
    return nc


def _q8(a):
    return np.clip(a, -448.0, 448.0).astype(E4NP)


def _pack_w(w, scale):
    """[out, K] weight -> [K/256, 128, 2, out] fp8 DoubleRow pair layout."""
    K = w.shape[1]
    A = (w.T * scale).reshape(K // 256, 2, 128, w.shape[0])
    return _q8(np.ascontiguousarray(A.transpose(0, 2, 1, 3)))


def _pack_x(xg, scale, C):
    """[n, DIM] tokens -> [DIM/256, 128, 2, C] fp8 DoubleRow pair layout."""
    A = np.zeros((DIM, C), np.float32)
    A[:, :xg.shape[0]] = (xg * scale).T
    A = A.reshape(NP, 2, 128, C)
    return _q8(np.ascontiguousarray(A.transpose(0, 2, 1, 3)))


def _gate_host(xt, gate_w, gate_b):
    logits = xt.astype(np.float64) @ gate_w.astype(np.float64).T \
        + gate_b.astype(np.float64)
    m = logits.max(axis=-1, keepdims=True)
    p = np.exp(logits - m)
    scores = p / p.sum(axis=-1, keepdims=True)
    order = np.argsort(-scores, axis=1, kind="stable")
    top_i = order[:, :TOPK]
    top_w = (np.take_along_axis(scores, top_i, axis=1)
             * ROUTE_SCALE).astype(np.float32)
    return top_i, top_w


def run(inputs, trace=False):
    x = np.ascontiguousarray(np.asarray(inputs["x"], dtype=np.float32))
    gate_w = np.asarray(inputs["gate_w"], dtype=np.float32)
    gate_b = np.asarray(inputs["gate_b"], dtype=np.float32)
    w1 = np.asarray(inputs["w1"], dtype=np.float32)
    b1 = np.asarray(inputs["b1"], dtype=np.float32)
    w3 = np.asarray(inputs["w3"], dtype=np.float32)
    b3 = np.asarray(inputs["b3"], dtype=np.float32)
    w2 = np.asarray(inputs["w2"], dtype=np.float32)
    b2 = np.asarray(inputs["b2"], dtype=np.float32)
    ws1 = np.asarray(inputs["ws1"], dtype=np.float32)
    bs1 = np.asarray(inputs["bs1"], dtype=np.float32)
    ws3 = np.asarray(inputs["ws3"], dtype=np.float32)
    bs3 = np.asarray(inputs["bs3"], dtype=np.float32)
    ws2 = np.asarray(inputs["ws2"], dtype=np.float32)
    bs2 = np.asarray(inputs["bs2"], dtype=np.float32)

    xt = x.reshape(T, DIM)
    top_i, top_w = _gate_host(xt, gate_w, gate_b)

    idx, wgt = [], []
    for e in range(E):
        toks = np.nonzero((top_i == e).any(axis=1))[0]
        idx.append(toks)
        slot = (top_i[toks] == e)
        wgt.append(top_w[toks][slot])

    cmax = max(len(i) for i in idx)
    C = max(256, -(-cmax // 32) * 32)

    # fp8 scales: per-tensor for x, per-expert per-tensor for weights; the
    # h scale comes from a 32-token fp32 sample of the true h distribution.
    sx = 16.0 / max(xt.std(), 1e-30)
    xprobe = xt[:32]
    s1 = np.empty(E, np.float64); s3 = np.empty(E, np.float64)
    s2 = np.empty(E, np.float64); sh = np.empty(E, np.float64)
    for e in range(E):
        s1[e] = 16.0 / max(w1[e].std(), 1e-30)
        s3[e] = 16.0 / max(w3[e].std(), 1e-30)
        s2[e] = 16.0 / max(w2[e].std(), 1e-30)
        a = xprobe @ w1[e].T + b1[e]
        bb = xprobe @ w3[e].T + b3[e]
        h = a / (1.0 + np.exp(-a)) * bb
        sh[e] = 8.0 / max(h.std(), 1e-30)

    ws1p = np.ascontiguousarray(
        ws1.T.reshape(ND, 128, SHARED_INTER)).astype(BFNP)
    ws3p = np.ascontiguousarray(
        ws3.T.reshape(ND, 128, SHARED_INTER)).astype(BFNP)
    ws2p = np.ascontiguousarray(ws2.T.reshape(NS, 128, DIM)).astype(BFNP)

    in_maps = []
    for e in range(E):
        scbuf = np.zeros((128, 68), np.float32)
        scbuf[:, 0:8] = b1[e].reshape(8, 128).T
        scbuf[:, 8:16] = (b3[e] * sh[e]).reshape(8, 128).T
        scbuf[:, 16:24] = b2[e].reshape(8, 128).T
        scbuf[:, 24:40] = bs1.reshape(16, 128).T
        scbuf[:, 40:56] = bs3.reshape(16, 128).T
        scbuf[:, 56:64] = bs2.reshape(8, 128).T
        scbuf[:, 64] = 1.0 / (sx * s1[e])
        scbuf[:, 65] = sh[e] / (sx * s3[e])
        scbuf[:, 66] = 1.0 / (sh[e] * s2[e])
        sl = slice(TS * e, TS * (e + 1))
        in_maps.append({
            "xe8": _pack_x(xt[idx[e]], sx, C),
            "w18": _pack_w(w1[e], s1[e]),
            "w38": _pack_w(w3[e], s3[e]),
            "w28": _pack_w(w2[e], s2[e]),
            "xs": np.ascontiguousarray(
                xt[sl].T.reshape(ND, 128, TS)).astype(BFNP),
            "ws1": ws1p, "ws3": ws3p, "ws2": ws2p,
            "scb": scbuf,
        })

    if C not in _program_cache:
        _program_cache[C] = build_program(C)
    nc = _program_cache[C]

    res = bass_utils.run_bass_kernel_spmd(
        nc, in_maps, core_ids=list(range(N_CORES)), trace=trace)

    y = np.empty((T, DIM), np.float32)
    for e in range(E):
        sl = slice(TS * e, TS * (e + 1))
        y[sl] = res.results[e]["ys"].reshape(DIM, TS).T
    for e in range(E):
        yee = res.results[e]["ye"].reshape(DIM, C).astype(np.float32)
        y[idx[e]] += yee[:, :len(idx[e])].T * wgt[e][:, None]
    return y.reshape(B, S, DIM), res


def kernel(**inputs) -> np.ndarray:
    out, _ = run(inputs, trace=False)
    return out


# revision 7
# speedup vs baseline: 1.4779x; 1.0063x over previous
"""MoE (DeepSeek-style) routed+shared expert forward on 8 TRN2 NeuronCores.

Strategy (expert-parallel, host-side dispatch):
  - Host computes the gate (softmax + top-2) in float64 and gathers each
    expert's routed tokens; core e processes expert e's tokens (padded to
    capacity C) plus a 1/8 slice of all tokens through the replicated
    shared-expert MLP.
  - Routed expert runs in fp8(e4m3) with DoubleRow matmuls (2x PE rate).
    Host quantizes x and the expert weights with per-tensor scales; the
    dequant scales fold into the activation/vector ops. Error budget: the
    routed path carries only ~23% of the output norm (gate weights are
    softmax scores ~0.2), so fp8's ~6% relative error lands at ~1.6e-2
    overall, within the 2e-2 tolerance.
  - Shared expert (97% of the output norm) stays bf16.
  - All weights are SBUF-resident; inputs arrive via a handful of large
    DMAs (one per operand) ordered to match PE consumption, with the
    shared-expert weights trickled in across the routed block loop so
    routed output DMAs never sit behind a multi-MB preload backlog.
  - Warmup matmuls on scratch SBUF run during the input DMA so the PE's
    DVFS clock is fully ramped when real work arrives.
"""

import sys

if "/opt/trn_rl_repo" not in sys.path:
    sys.path.insert(0, "/opt/trn_rl_repo")

import ml_dtypes
import numpy as np

import concourse.bass as bass
import concourse.tile as tile
from concourse import bacc, mybir
from concourse import bass_utils
from concourse.alu_op_type import AluOpType

B, S, DIM = 4, 2048, 1024
T = B * S
INTER = 1024
E = 8
TOPK = 2
ROUTE_SCALE = 1.0
SHARED_INTER = 2048
N_CORES = 8
TS = T // N_CORES  # shared-expert tokens per core
BLK = 512
N_WARM = 16

F32 = mybir.dt.float32
BF16 = mybir.dt.bfloat16
FP8 = mybir.dt.float8e4
SILU = mybir.ActivationFunctionType.Silu
IDENT = mybir.ActivationFunctionType.Identity
DR = mybir.MatmulPerfMode.DoubleRow
MUL = AluOpType.mult
ADD = AluOpType.add

E4NP = ml_dtypes.float8_e4m3fn
BFNP = ml_dtypes.bfloat16

ND = DIM // 128           # 8 k-tiles over DIM
NP = ND // 2              # 4 DoubleRow k-pair tiles over DIM
NI = INTER // 128         # 8 tiles over INTER
NS = SHARED_INTER // 128  # 16 tiles over SHARED_INTER

_program_cache = {}


def _blocks(total):
    """Split into <=512-wide even blocks of near-equal size (all >=256 so
    per-instruction LDWEIGHTS overhead stays hidden)."""
    nb = -(-total // BLK)
    b = -(-total // (nb * 32)) * 32
    sizes = [b] * (nb - 1) + [total - b * (nb - 1)]
    assert all(256 <= s <= BLK and s % 2 == 0 for s in sizes), sizes
    out, o = [], 0
    for s in sizes:
        out.append((o, s))
        o += s
    return out


def build_program(C):
    nc = bacc.Bacc("TRN2", target_bir_lowering=False, debug=False,
                   num_devices=N_CORES)

    def din(name, shape, dt):
        return nc.dram_tensor(name, shape, dt, kind="ExternalInput").ap()

    def dout(name, shape, dt):
        return nc.dram_tensor(name, shape, dt, kind="ExternalOutput").ap()

    xe8 = din("xe8", (128, ND, C), FP8)        # routed tokens, fp8 pairs
    w18 = din("w18", (128, ND, INTER), FP8)    # w1[e].T in DR pair layout
    w38 = din("w38", (128, ND, INTER), FP8)
    w28 = din("w28", (128, ND, DIM), FP8)      # w2[e].T in DR pair layout
    xs = din("xs", (128, ND, TS), BF16)        # shared-token slice
    ws1 = din("ws1", (128, ND, SHARED_INTER), BF16)
    ws3 = din("ws3", (128, ND, SHARED_INTER), BF16)
    ws2 = din("ws2", (128, NS, DIM), BF16)
    scb = din("scb", (128, 68), F32)  # packed biases + dequant scales
    ye = dout("ye", (ND, 128, C), BF16)
    ys = dout("ys", (ND, 128, TS), BF16)

    rblocks = _blocks(C)
    sblocks = _blocks(TS)

    with tile.TileContext(nc) as tc:
        with tc.tile_pool(name="const", bufs=1) as cpool, \
             tc.tile_pool(name="tmp", bufs=2) as tpool, \
             tc.tile_pool(name="hr", bufs=2) as hrpool, \
             tc.tile_pool(name="hsh", bufs=2) as hspool, \
             tc.tile_pool(name="yout", bufs=6) as ypool, \
             tc.tile_pool(name="ysout", bufs=2) as yspool, \
             tc.tile_pool(name="ps", bufs=2, space="PSUM") as pspool, \
             tc.tile_pool(name="wps", bufs=1, space="PSUM") as wpspool:

            # ---- PE warmup: ramp the DVFS clock while input DMA runs ----
            wsc = cpool.tile([128, 416], BF16, tag="warm")
            nc.vector.memset(wsc[:], 0.25)
            wps = wpspool.tile([128, 416], F32, tag="wps",
                               padded_shape=[128, BLK])
            for _ in range(N_WARM):
                nc.tensor.matmul(wps[:], wsc[:, 0:128], wsc[:],
                                 start=True, stop=True)

            # ---- input DMAs, large transfers in PE consumption order ----
            ball = cpool.tile([128, 68], F32, tag="scb")
            nc.sync.dma_start(ball[:], scb[:])
            b1c = lambda mi: ball[:, mi:mi + 1]
            b3c = lambda mi: ball[:, 8 + mi:9 + mi]
            b2c = lambda md: ball[:, 16 + md:17 + md]
            bs1c = lambda mi: ball[:, 24 + mi:25 + mi]
            bs3c = lambda mi: ball[:, 40 + mi:41 + mi]
            bs2c = lambda md: ball[:, 56 + md:57 + md]
            sc1 = ball[:, 64:65]
            sc3 = ball[:, 65:66]
            scy = ball[:, 66:67]

            w1_t = cpool.tile([128, ND, INTER], FP8, tag="w1")
            nc.sync.dma_start(w1_t[:], w18[:])
            xe_t = cpool.tile([128, ND, C], FP8, tag="xe", name="xe")
            off0, n0 = rblocks[0]
            nc.sync.dma_start(xe_t[:, :, off0:off0 + n0],
                              xe8[:, :, off0:off0 + n0])
            w3_t = cpool.tile([128, ND, INTER], FP8, tag="w3")
            nc.sync.dma_start(w3_t[:], w38[:])
            w2_t = cpool.tile([128, ND, DIM], FP8, tag="w2")
            nc.sync.dma_start(w2_t[:], w28[:])
            for (off, n) in rblocks[1:]:
                nc.sync.dma_start(xe_t[:, :, off:off + n],
                                  xe8[:, :, off:off + n])
            xs_t = cpool.tile([128, ND, TS], BF16, tag="xs", name="xs")
            nc.sync.dma_start(xs_t[:], xs[:])
            ws1_t = cpool.tile([128, ND, SHARED_INTER], BF16, tag="ws1",
                               name="ws1")
            ws3_t = cpool.tile([128, ND, SHARED_INTER], BF16, tag="ws3",
                               name="ws3")
            ws2_t = cpool.tile([128, NS, DIM], BF16, tag="ws2", name="ws2")
            # shared-weight DMAs trickled across the routed block loop in
            # ~2MB chunks so routed output DMAs never queue behind them
            hsi = SHARED_INTER // 2
            ws_dmas = [
                (ws1_t[:, :, 0:hsi], ws1[:, :, 0:hsi]),
                (ws1_t[:, :, hsi:SHARED_INTER], ws1[:, :, hsi:SHARED_INTER]),
                (ws3_t[:, :, 0:hsi], ws3[:, :, 0:hsi]),
                (ws3_t[:, :, hsi:SHARED_INTER], ws3[:, :, hsi:SHARED_INTER]),
                (ws2_t[:, 0:NS // 2, :], ws2[:, 0:NS // 2, :]),
                (ws2_t[:, NS // 2:NS, :], ws2[:, NS // 2:NS, :]),
            ]
            nws = -(-len(ws_dmas) // len(rblocks))

            # ---- Phase 1: routed expert, fp8 DoubleRow ----
            for bi, (off, n) in enumerate(rblocks):
                for dst, src in ws_dmas[bi * nws:(bi + 1) * nws]:
                    nc.sync.dma_start(dst, src)
                h8 = hrpool.tile([128, ND, n], FP8, tag="h8", name="h8",
                                 padded_shape=[128, ND, BLK])
                for mi in range(NI):
                    ps1 = pspool.tile([128, n], F32, tag="ps1",
                                      padded_shape=[128, BLK])
                    for j in range(NP):
                        nc.tensor.matmul(
                            ps1[:], w1_t[:, 2 * j:2 * j + 2,
                                         mi * 128:(mi + 1) * 128],
                            xe_t[:, 2 * j:2 * j + 2, off:off + n],
                            start=(j == 0), stop=(j == NP - 1),
                            perf_mode=DR)
                    ps3 = pspool.tile([128, n], F32, tag="ps3",
                                      padded_shape=[128, BLK])
                    for j in range(NP):
                        nc.tensor.matmul(
                            ps3[:], w3_t[:, 2 * j:2 * j + 2,
                                         mi * 128:(mi + 1) * 128],
                            xe_t[:, 2 * j:2 * j + 2, off:off + n],
                            start=(j == 0), stop=(j == NP - 1),
                            perf_mode=DR)
                    t1 = tpool.tile([128, n], BF16, tag="t1", name="t1",
                                    padded_shape=[128, BLK])
                    nc.scalar.activation(t1[:], ps1[:], SILU,
                                         bias=b1c(mi), scale=sc1)
                    t3 = tpool.tile([128, n], BF16, tag="t3", name="t3",
                                    padded_shape=[128, BLK])
                    nc.vector.tensor_scalar(t3[:], ps3[:], sc3, b3c(mi),
                                            MUL, ADD)
                    nc.vector.tensor_mul(h8[:, mi, :], t1[:], t3[:])
                for md in range(ND):
                    psy = pspool.tile([128, n], F32, tag="psy",
                                      padded_shape=[128, BLK])
                    for j in range(NP):
                        nc.tensor.matmul(
                            psy[:], w2_t[:, 2 * j:2 * j + 2,
                                         md * 128:(md + 1) * 128],
                            h8[:, 2 * j:2 * j + 2, :],
                            start=(j == 0), stop=(j == NP - 1),
                            perf_mode=DR)
                    yt = ypool.tile([128, n], BF16, tag="yt", name="yt",
                                    padded_shape=[128, BLK])
                    nc.scalar.activation(yt[:], psy[:], IDENT,
                                         bias=b2c(md), scale=scy)
                    nc.sync.dma_start(ye[md][:, off:off + n], yt[:])

            # ---- Phase 2: shared expert, bf16 ----
            for (off, n) in sblocks:
                hs = [hspool.tile([128, n], BF16, tag=f"hs_{mi}",
                                  name=f"hs{mi}", padded_shape=[128, BLK])
                      for mi in range(NS)]
                for mi in range(NS):
                    ps1 = pspool.tile([128, n], F32, tag="ps1",
                                      padded_shape=[128, BLK])
                    for dk in range(ND):
                        nc.tensor.matmul(
                            ps1[:], ws1_t[:, dk, mi * 128:(mi + 1) * 128],
                            xs_t[:, dk, off:off + n],
                            start=(dk == 0), stop=(dk == ND - 1))
                    ps3 = pspool.tile([128, n], F32, tag="ps3",
                                      padded_shape=[128, BLK])
                    for dk in range(ND):
                        nc.tensor.matmul(
                            ps3[:], ws3_t[:, dk, mi * 128:(mi + 1) * 128],
                            xs_t[:, dk, off:off + n],
                            start=(dk == 0), stop=(dk == ND - 1))
                    t1 = tpool.tile([128, n], BF16, tag="t1", name="t1",
                                    padded_shape=[128, BLK])
                    nc.scalar.activation(t1[:], ps1[:], SILU, bias=bs1c(mi))
                    t3 = tpool.tile([128, n], BF16, tag="t3", name="t3",
                                    padded_shape=[128, BLK])
                    nc.vector.tensor_scalar(t3[:], ps3[:], 1.0, bs3c(mi),
                                            MUL, ADD)
                    nc.vector.tensor_mul(hs[mi][:], t1[:], t3[:])
                for md in range(ND):
                    psy = pspool.tile([128, n], F32, tag="psy",
                                      padded_shape=[128, BLK])
                    for mi in range(NS):
                        nc.tensor.matmul(
                            psy[:], ws2_t[:, mi, md * 128:(md + 1) * 128],
                            hs[mi][:],
                            start=(mi == 0), stop=(mi == NS - 1))
                    yts = yspool.tile([128, n], BF16, tag="yts", name="yts",
                                      padded_shape=[128, BLK])
                    nc.scalar.activation(yts[:], psy[:], IDENT, bias=bs2c(md))
                    nc.sync.dma_start(ys[md][:, off:off + n], yts[:])

    nc.compile()
    return nc


def _q8(a):
    return np.clip(a, -448.0, 448.0).astype(E4NP)


def _pack_w(w, scale):
    """[out, K] weight -> [128, K/128, out] fp8 DoubleRow pair layout
    (partition-major; k-subtile pairs adjacent in the middle dim)."""
    K = w.shape[1]
    A = (w.T * scale).reshape(K // 256, 2, 128, w.shape[0])
    return _q8(np.ascontiguousarray(
        A.transpose(2, 0, 1, 3).reshape(128, K // 128, w.shape[0])))


def _pack_x(xg, scale, C):
    """[n, DIM] tokens -> [128, DIM/128, C] fp8 DoubleRow pair layout."""
    A = np.zeros((DIM, C), np.float32)
    A[:, :xg.shape[0]] = (xg * scale).T
    A = A.reshape(NP, 2, 128, C)
    return _q8(np.ascontiguousarray(
        A.transpose(2, 0, 1, 3).reshape(128, ND, C)))


def _pack_bf(w_t, nk):
    """[K, M] (already transposed) -> [128, nk, M] bf16."""
    K, M = w_t.shape
    return np.ascontiguousarray(
        w_t.reshape(nk, 128, M).transpose(1, 0, 2)).astype(BFNP)


def _gate_host(xt, gate_w, gate_b):
    logits = xt.astype(np.float64) @ gate_w.astype(np.float64).T \
        + gate_b.astype(np.float64)
    m = logits.max(axis=-1, keepdims=True)
    p = np.exp(logits - m)
    scores = p / p.sum(axis=-1, keepdims=True)
    order = np.argsort(-scores, axis=1, kind="stable")
    top_i = order[:, :TOPK]
    top_w = (np.take_along_axis(scores, top_i, axis=1)
             * ROUTE_SCALE).astype(np.float32)
    return top_i, top_w


def run(inputs, trace=False):
    x = np.ascontiguousarray(np.asarray(inputs["x"], dtype=np.float32))
    gate_w = np.asarray(inputs["gate_w"], dtype=np.float32)
    gate_b = np.asarray(inputs["gate_b"], dtype=np.float32)
    w1 = np.asarray(inputs["w1"], dtype=np.float32)
    b1 = np.asarray(inputs["b1"], dtype=np.float32)
    w3 = np.asarray(inputs["w3"], dtype=np.float32)
    b3 = np.asarray(inputs["b3"], dtype=np.float32)
    w2 = np.asarray(inputs["w2"], dtype=np.float32)
    b2 = np.asarray(inputs["b2"], dtype=np.float32)
    ws1 = np.asarray(inputs["ws1"], dtype=np.float32)
    bs1 = np.asarray(inputs["bs1"], dtype=np.float32)
    ws3 = np.asarray(inputs["ws3"], dtype=np.float32)
    bs3 = np.asarray(inputs["bs3"], dtype=np.float32)
    ws2 = np.asarray(inputs["ws2"], dtype=np.float32)
    bs2 = np.asarray(inputs["bs2"], dtype=np.float32)

    xt = x.reshape(T, DIM)
    top_i, top_w = _gate_host(xt, gate_w, gate_b)

    idx, wgt = [], []
    for e in range(E):
        toks = np.nonzero((top_i == e).any(axis=1))[0]
        idx.append(toks)
        slot = (top_i[toks] == e)
        wgt.append(top_w[toks][slot])

    cmax = max(len(i) for i in idx)
    C = max(256, -(-cmax // 32) * 32)

    # fp8 scales: per-tensor for x, per-expert per-tensor for weights; the
    # h scale comes from a 32-token fp32 sample of the true h distribution.
    sx = 16.0 / max(xt.std(), 1e-30)
    xprobe = xt[:32]
    s1 = np.empty(E, np.float64); s3 = np.empty(E, np.float64)
    s2 = np.empty(E, np.float64); sh = np.empty(E, np.float64)
    for e in range(E):
        s1[e] = 16.0 / max(w1[e].std(), 1e-30)
        s3[e] = 16.0 / max(w3[e].std(), 1e-30)
        s2[e] = 16.0 / max(w2[e].std(), 1e-30)
        a = xprobe @ w1[e].T + b1[e]
        bb = xprobe @ w3[e].T + b3[e]
        h = a / (1.0 + np.exp(-a)) * bb
        sh[e] = 8.0 / max(h.std(), 1e-30)

    ws1p = _pack_bf(ws1.T, ND)
    ws3p = _pack_bf(ws3.T, ND)
    ws2p = _pack_bf(ws2.T, NS)

    in_maps = []
    for e in range(E):
        scbuf = np.zeros((128, 68), np.float32)
        scbuf[:, 0:8] = b1[e].reshape(8, 128).T
        scbuf[:, 8:16] = (b3[e] * sh[e]).reshape(8, 128).T
        scbuf[:, 16:24] = b2[e].reshape(8, 128).T
        scbuf[:, 24:40] = bs1.reshape(16, 128).T
        scbuf[:, 40:56] = bs3.reshape(16, 128).T
        scbuf[:, 56:64] = bs2.reshape(8, 128).T
        scbuf[:, 64] = 1.0 / (sx * s1[e])
        scbuf[:, 65] = sh[e] / (sx * s3[e])
        scbuf[:, 66] = 1.0 / (sh[e] * s2[e])
        sl = slice(TS * e, TS * (e + 1))
        in_maps.append({
            "xe8": _pack_x(xt[idx[e]], sx, C),
            "w18": _pack_w(w1[e], s1[e]),
            "w38": _pack_w(w3[e], s3[e]),
            "w28": _pack_w(w2[e], s2[e]),
            "xs": _pack_bf(xt[sl].T, ND),
            "ws1": ws1p, "ws3": ws3p, "ws2": ws2p,
            "scb": scbuf,
        })

    if C not in _program_cache:
        _program_cache[C] = build_program(C)
    nc = _program_cache[C]

    res = bass_utils.run_bass_kernel_spmd(
        nc, in_maps, core_ids=list(range(N_CORES)), trace=trace)

    y = np.empty((T, DIM), np.float32)
    for e in range(E):
        sl = slice(TS * e, TS * (e + 1))
        y[sl] = res.results[e]["ys"].reshape(DIM, TS).T.astype(np.float32)
    for e in range(E):
        yee = res.results[e]["ye"].reshape(DIM, C).astype(np.float32)
        y[idx[e]] += yee[:, :len(idx[e])].T * wgt[e][:, None]
    return y.reshape(B, S, DIM), res


def kernel(**inputs) -> np.ndarray:
    out, _ = run(inputs, trace=False)
    return out
